# revision 4
# baseline (speedup 1.0000x reference)
"""DoubleAttention forward on 8 Trainium2 NeuronCores.

Reference (per sample, x: [512, 4096] after flattening h*w):
    A = wA @ x + bA            [128, n]
    B = wB @ x + bB            [128, n]
    V = wV @ x + bV            [128, n]
    M = softmax(B, axis=ch)    [128, n]
    W = softmax(V, axis=ch)    [128, n]
    gd = A @ M.T               [128, 128]
    Z = gd @ W                 [128, n]
    out = wR @ Z + bR          [512, n]

Sharding: data-parallel over batch, 16 samples -> 8 cores x 2.

Layout strategy (per sample, n tiled by 128):
  phase 1: P1[n,384] = x_chunk.T @ [wA.T|wB.T|wV.T]  (4 K-chunks of 128,
           fp32r, moving dim 384) -> softmax over the free dim (exp on
           ACT with fused row-sum, per-partition scale on DVE);
           gdT[k,m] += MT.T @ AT on PE across the 32 tiles.
  phase 3: PE-transpose WT tiles back to [k,n]; Zun[m,n] = gdT.T @ W;
           out[c,n] = wRT.T @ Zun (4 chunks); bias + copy on ACT/DVE;
           1 MB DMA per 512-column group.
Biases enter as rank-1 PSUM-accumulate matmuls (ones[1,n] x bias[1,384])
and are skipped entirely when the bias vectors are all zero.
"""

import sys

if "/opt/trn_rl_repo" not in sys.path:
    sys.path.insert(0, "/opt/trn_rl_repo")

import numpy as np

import concourse.bacc as bacc
import concourse.tile as tile
from concourse import masks, mybir
from concourse.bass_utils import run_bass_kernel_spmd

N_CORES = 8
B_GLOBAL = 16
B_LOC = B_GLOBAL // N_CORES
C_IN, C_M, C_N = 512, 128, 128
H = W = 64
N = H * W                      # 4096 spatial positions
NT = N // 128                  # 32 tiles of 128 positions
KC = C_IN // 128               # 4 contraction chunks
NG = N // 512                  # 8 output groups of 512 positions
F32 = mybir.dt.float32
F32R = mybir.dt.float32r


def _build(has_bias_abv: bool, has_bias_r: bool):
    nc = bacc.Bacc("TRN2", target_bir_lowering=False, debug=False)

    x_d = nc.dram_tensor("x", (B_LOC, C_IN, N), F32R, kind="ExternalInput")
    wcat_d = nc.dram_tensor("wcat", (KC, 128, 384), F32R, kind="ExternalInput")
    wrt_d = nc.dram_tensor("wrt", (128, C_IN), F32R, kind="ExternalInput")
    if has_bias_abv:
        bcat_d = nc.dram_tensor("bcat", (1, 384), F32R, kind="ExternalInput")
    if has_bias_r:
        brt_d = nc.dram_tensor("brt", (128, KC), F32, kind="ExternalInput")
    out_d = nc.dram_tensor("out", (B_LOC, C_IN, N), F32, kind="ExternalOutput")

    with tile.TileContext(nc) as tc:
        with (
            tc.tile_pool(name="const", bufs=1) as constp,
            tc.tile_pool(name="xs", bufs=7) as xsp,
            tc.tile_pool(name="wt", bufs=2) as wtp,
            tc.tile_pool(name="at", bufs=3) as atp,
            tc.tile_pool(name="mt", bufs=3) as mtp,
            tc.tile_pool(name="st", bufs=4) as stp,
            tc.tile_pool(name="gds", bufs=2) as gdsp,
            tc.tile_pool(name="wsb", bufs=2) as wsbp,
            tc.tile_pool(name="zsb", bufs=2) as zsbp,
            tc.tile_pool(name="osb", bufs=2) as osbp,
            tc.tile_pool(name="pP", bufs=2, space="PSUM") as pP,
            tc.tile_pool(name="pG", bufs=1, space="PSUM") as pG,
            tc.tile_pool(name="pW", bufs=2, space="PSUM") as pW,
            tc.tile_pool(name="pZ", bufs=1, space="PSUM") as pZ,
            tc.tile_pool(name="pO", bufs=2, space="PSUM") as pO,
        ):
            wcat = constp.tile([128, KC, 384], F32R)
            nc.sync.dma_start(wcat[:], wcat_d.ap().rearrange("k p j -> p k j"))
            wrt = constp.tile([128, C_IN], F32R)
            nc.sync.dma_start(wrt[:], wrt_d[:])
            ident = constp.tile([128, 128], F32)
            masks.make_identity(nc, ident[:])
            if has_bias_abv:
                bcat = constp.tile([1, 384], F32R)
                nc.sync.dma_start(bcat[:], bcat_d[:])
                ones1 = constp.tile([1, 128], F32R)
                nc.gpsimd.memset(ones1[:], 1.0)
            if has_bias_r:
                brt = constp.tile([128, KC], F32)
                nc.sync.dma_start(brt[:], brt_d[:])

            for s in range(B_LOC):
                # ---- phase 1: projections + channel softmax + gdT ----
                xs = [
                    xsp.tile([128, N], F32R, tag="xs", name=f"xs{s}_{k}")
                    for k in range(KC)
                ]
                for k in range(KC):
                    nc.sync.dma_start(xs[k][:], x_d[s, k * 128:(k + 1) * 128, :])

                wts = wtp.tile([128, N], F32)      # normalized V^T store
                gdt = pG.tile([128, 128], F32)     # gd^T accumulator
                pend = None                        # (MT, AT) awaiting gd matmul

                for nt in range(NT):
                    p1 = pP.tile([128, 384], F32)
                    sl = slice(nt * 128, (nt + 1) * 128)
                    for k in range(KC):
                        nc.tensor.matmul(
                            p1[:],
                            xs[k][:, sl],
                            wcat[:, k, :],
                            start=(k == 0),
                            stop=(k == KC - 1 and not has_bias_abv),
                        )
                    if has_bias_abv:
                        nc.tensor.matmul(
                            p1[:], ones1[:], bcat[:], start=False, stop=True
                        )

                    # gd matmul for the previous tile goes here so the PE has
                    # dense work while this tile's softmax completes.
                    if pend is not None:
                        pmt, pat = pend
                        nc.tensor.matmul(
                            gdt[:], pmt[:], pat[:],
                            start=(nt == 1), stop=False, skip_group_check=True,
                        )

                    at = atp.tile([128, 128], F32R)
                    nc.vector.tensor_copy(at[:], p1[:, 0:128])
                    mtr = mtp.tile([128, 128], F32, tag="mtr")
                    sb = stp.tile([128, 1], F32, tag="sb")
                    nc.scalar.activation(
                        mtr[:], p1[:, 128:256],
                        mybir.ActivationFunctionType.Exp, accum_out=sb[:],
                    )
                    wtr = mtp.tile([128, 128], F32, tag="wtr")
                    sv = stp.tile([128, 1], F32, tag="sv")
                    nc.scalar.activation(
                        wtr[:], p1[:, 256:384],
                        mybir.ActivationFunctionType.Exp, accum_out=sv[:],
                    )
                    rb = stp.tile([128, 1], F32, tag="rb")
                    nc.vector.reciprocal(rb[:], sb[:])
                    rv = stp.tile([128, 1], F32, tag="rv")
                    nc.vector.reciprocal(rv[:], sv[:])
                    mt = mtp.tile([128, 128], F32R, tag="mt")
                    nc.vector.tensor_scalar_mul(mt[:], mtr[:], rb[:])
                    nc.vector.tensor_scalar_mul(wts[:, sl], wtr[:], rv[:])
                    pend = (mt, at)

                pmt, pat = pend
                nc.tensor.matmul(
                    gdt[:], pmt[:], pat[:],
                    start=(NT == 1), stop=True, skip_group_check=True,
                )
                gdts = gdsp.tile([128, 128], F32R)
                nc.vector.tensor_copy(gdts[:], gdt[:])

                # ---- phase 3: distribute + reconstruct, one lag group ----
                wtrans = []
                for g in range(NG):
                    wpt = pW.tile([128, 512], F32)
                    for j in range(KC):
                        nc.tensor.transpose(
                            wpt[:, j * 128:(j + 1) * 128],
                            wts[:, (4 * g + j) * 128:(4 * g + j + 1) * 128],
                            ident[:],
                        )
                    wsb = wsbp.tile([128, 512], F32R)
                    nc.vector.tensor_copy(wsb[:], wpt[:])
                    wtrans.append(wsb)

                    if g > 0:
                        _emit_group(nc, s, g - 1, wtrans[g - 1], gdts, wrt,
                                    brt if has_bias_r else None,
                                    pZ, pO, zsbp, osbp, out_d, has_bias_r)
                _emit_group(nc, s, NG - 1, wtrans[NG - 1], gdts, wrt,
                            brt if has_bias_r else None,
                            pZ, pO, zsbp, osbp, out_d, has_bias_r)

    nc.compile()
    return nc


def _emit_group(nc, s, g, wsb, gdts, wrt, brt, pZ, pO, zsbp, osbp, out_d,
                has_bias_r):
    zun = pZ.tile([128, 512], F32)
    nc.tensor.matmul(zun[:], gdts[:], wsb[:], start=True, stop=True)
    zsb = zsbp.tile([128, 512], F32R)
    nc.scalar.copy(zsb[:], zun[:])
    osb = osbp.tile([128, KC, 512], F32)
    for k in range(KC):
        ock = pO.tile([128, 512], F32)
        nc.tensor.matmul(
            ock[:], wrt[:, k * 128:(k + 1) * 128], zsb[:],
            start=True, stop=True,
        )
        if has_bias_r:
            nc.scalar.activation(
                osb[:, k, :], ock[:],
                mybir.ActivationFunctionType.Identity, bias=brt[:, k:k + 1],
            )
        elif k % 2 == 0:
            nc.scalar.copy(osb[:, k, :], ock[:])
        else:
            nc.vector.tensor_copy(osb[:, k, :], ock[:])
    dst = out_d[s].rearrange("(k p) n -> p k n", p=128)[:, :, g * 512:(g + 1) * 512]
    nc.sync.dma_start(dst, osb[:])


_CACHE = {}


def _get_nc(has_bias_abv: bool, has_bias_r: bool):
    key = (has_bias_abv, has_bias_r)
    if key not in _CACHE:
        _CACHE[key] = _build(*key)
    return _CACHE[key]


def _run(inputs, trace=False, **spmd_kwargs):
    x = np.ascontiguousarray(np.asarray(inputs["x"], dtype=np.float32))
    b, c, h, w = x.shape
    assert (b, c, h, w) == (B_GLOBAL, C_IN, H, W), x.shape
    wA = np.asarray(inputs["wA"], np.float32)
    wB = np.asarray(inputs["wB"], np.float32)
    wV = np.asarray(inputs["wV"], np.float32)
    wR = np.asarray(inputs["wR"], np.float32)
    bA = np.asarray(inputs["bA"], np.float32)
    bB = np.asarray(inputs["bB"], np.float32)
    bV = np.asarray(inputs["bV"], np.float32)
    bR = np.asarray(inputs["bR"], np.float32)

    has_bias_abv = bool(np.any(bA) or np.any(bB) or np.any(bV))
    has_bias_r = bool(np.any(bR))
    nc = _get_nc(has_bias_abv, has_bias_r)

    # [KC, 128, 384] : chunk k holds [wA.T | wB.T | wV.T][k*128:(k+1)*128, :]
    wcat = np.concatenate([wA.T, wB.T, wV.T], axis=1).reshape(KC, 128, 3 * 128)
    wcat = np.ascontiguousarray(wcat)
    wrt = np.ascontiguousarray(wR.T)                      # [128, 512]
    base = {"wcat": wcat, "wrt": wrt}
    if has_bias_abv:
        base["bcat"] = np.concatenate([bA, bB, bV])[None, :].copy()
    if has_bias_r:
        base["brt"] = np.ascontiguousarray(bR.reshape(KC, 128).T)

    xf = x.reshape(B_GLOBAL, C_IN, N)
    in_maps = [
        dict(base, x=np.ascontiguousarray(xf[ci * B_LOC:(ci + 1) * B_LOC]))
        for ci in range(N_CORES)
    ]
    res = run_bass_kernel_spmd(
        nc, in_maps, core_ids=list(range(N_CORES)), trace=trace, **spmd_kwargs
    )
    out = np.concatenate([res.results[ci]["out"] for ci in range(N_CORES)], axis=0)
    return out.reshape(B_GLOBAL, C_IN, H, W), res


def kernel(**inputs):
    out, _ = _run(inputs)
    return out


# revision 5
# speedup vs baseline: 1.4862x; 1.4862x over previous
"""DoubleAttention forward on 8 Trainium2 NeuronCores.

Reference (per sample, x: [512, 4096] after flattening h*w):
    A = wA @ x + bA            [128, n]
    B = wB @ x + bB            [128, n]
    V = wV @ x + bV            [128, n]
    M = softmax(B, axis=ch)    [128, n]
    W = softmax(V, axis=ch)    [128, n]
    gd = A @ M.T               [128, 128]
    Z = gd @ W                 [128, n]
    out = wR @ Z + bR          [512, n]

Sharding: data-parallel over batch, 16 samples -> 8 cores x 2 each.

Implementation notes:
  - All matmul inputs are fp16 (exact products, fp32 PSUM accumulation);
    x / weights are converted host-side, halving the input DMA as well.
  - Transposed layout: per 128-wide n-tile, P1[n, A|B|V] = x_chunk.T @
    [wA.T|wB.T|wV.T], so the channel softmax is a free-dim op.
  - n-tiles processed in PAIRS sharing one 2-bank PSUM tile so softmax
    elementwise ops run at 2x width (amortizes fixed per-op cost).
  - gd^T accumulates on PE across tiles (2-pair emission lag keeps PE
    dense while softmax of the newest pair drains).
  - Phase 3 transposes normalized W^T tiles back via PE, then
    Z = gdT.T @ W and out = wRT.T @ Z (all fp16, N=512 moving dim).
  - Output staged fp16 and upcast host-side (halves the store DMA).
  - Biases fold in as rank-1 PSUM-accumulate matmuls / ACT bias adds,
    all skipped when the bias vectors are zero (the common case).
"""

import sys

if "/opt/trn_rl_repo" not in sys.path:
    sys.path.insert(0, "/opt/trn_rl_repo")

import numpy as np

import concourse.bacc as bacc
import concourse.tile as tile
from concourse import masks, mybir
from concourse.bass_utils import run_bass_kernel_spmd

N_CORES = 8
B_GLOBAL = 16
B_LOC = B_GLOBAL // N_CORES
C_IN, C_M, C_N = 512, 128, 128
H = W = 64
N = H * W                      # 4096 spatial positions
NT = N // 128                  # 32 tiles of 128 positions
NP = NT // 2                   # 16 tile-pairs
KC = C_IN // 128               # 4 contraction chunks
NG = N // 512                  # 8 output groups of 512 positions
F32 = mybir.dt.float32
F16 = mybir.dt.float16
EXP = mybir.ActivationFunctionType.Exp
IDENT = mybir.ActivationFunctionType.Identity


def _build(has_bias_abv: bool, has_bias_r: bool):
    nc = bacc.Bacc("TRN2", target_bir_lowering=False, debug=False)

    x_d = nc.dram_tensor("x", (B_LOC, C_IN, N), F16, kind="ExternalInput")
    wcat_d = nc.dram_tensor("wcat", (KC, 128, 384), F16, kind="ExternalInput")
    wrt_d = nc.dram_tensor("wrt", (128, C_IN), F16, kind="ExternalInput")
    if has_bias_abv:
        bcat_d = nc.dram_tensor("bcat", (1, 384), F16, kind="ExternalInput")
    if has_bias_r:
        brt_d = nc.dram_tensor("brt", (128, KC), F32, kind="ExternalInput")
    out_d = nc.dram_tensor("out", (B_LOC, C_IN, N), F16, kind="ExternalOutput")

    with tile.TileContext(nc) as tc:
        with (
            tc.tile_pool(name="const", bufs=1) as constp,
            tc.tile_pool(name="xq", bufs=8) as xqp,
            tc.tile_pool(name="mw", bufs=2 * NP) as mwp,
            tc.tile_pool(name="at", bufs=5) as atp,
            tc.tile_pool(name="ex", bufs=3) as exp_,
            tc.tile_pool(name="st", bufs=4) as stp,
            tc.tile_pool(name="gds", bufs=2) as gdsp,
            tc.tile_pool(name="wsb", bufs=3) as wsbp,
            tc.tile_pool(name="zsb", bufs=2) as zsbp,
            tc.tile_pool(name="osb", bufs=2) as osbp,
            tc.tile_pool(name="pP", bufs=2, space="PSUM") as pP,
            tc.tile_pool(name="pG", bufs=1, space="PSUM") as pG,
            tc.tile_pool(name="pW", bufs=1, space="PSUM") as pW,
            tc.tile_pool(name="pZO", bufs=2, space="PSUM") as pZO,
        ):
            wcat = constp.tile([128, KC, 384], F16)
            nc.sync.dma_start(wcat[:], wcat_d.ap().rearrange("k p j -> p k j"))
            wrt = constp.tile([128, C_IN], F16)
            nc.sync.dma_start(wrt[:], wrt_d[:])
            ident16 = constp.tile([128, 128], F16)
            masks.make_identity(nc, ident16[:])
            if has_bias_abv:
                bcat = constp.tile([1, 384], F16)
                nc.sync.dma_start(bcat[:], bcat_d[:])
                ones1 = constp.tile([1, 128], F16)
                nc.gpsimd.memset(ones1[:], 1.0)
            if has_bias_r:
                brt = constp.tile([128, KC], F32)
                nc.sync.dma_start(brt[:], brt_d[:])

            for s in range(B_LOC):
                # ---- phase 1: projections + channel softmax + gdT ----
                xq = [
                    xqp.tile([128, KC, 1024], F16, tag="xq", name=f"xq{s}_{q}")
                    for q in range(KC)
                ]
                for q in range(KC):
                    src = x_d[s].rearrange("(k p) n -> p k n", p=128)
                    nc.sync.dma_start(
                        xq[q][:], src[:, :, q * 1024:(q + 1) * 1024]
                    )

                mwts = [
                    mwp.tile([128, 2, 2, 128], F16, tag="mw", name=f"mw{s}_{i}")
                    for i in range(NP)
                ]
                ats = {}
                gdt = pG.tile([128, 128], F32)

                def emit_gd(i, first, last):
                    for j in (0, 1):
                        nc.tensor.matmul(
                            gdt[:], mwts[i][:, j, 0, :], ats[i][:, j, :],
                            start=(first and j == 0), stop=(last and j == 1),
                            skip_group_check=True,
                        )

                for i in range(NP):
                    p1 = pP.tile([128, 1024], F32)
                    p1v = p1.rearrange("p (j r c) -> p j r c", j=2, c=128)
                    for j in (0, 1):
                        nt = 2 * i + j
                        q, col = nt // 8, (nt % 8) * 128
                        dst = p1[:, j * 512:j * 512 + 384]
                        for k in range(KC):
                            nc.tensor.matmul(
                                dst, xq[q][:, k, col:col + 128], wcat[:, k, :],
                                start=(k == 0),
                                stop=(k == KC - 1 and not has_bias_abv),
                            )
                        if has_bias_abv:
                            nc.tensor.matmul(
                                dst, ones1[:], bcat[:], start=False, stop=True
                            )

                    # gd for pair i-2: keeps PE busy while softmax(i-1..i) runs
                    if i >= 2:
                        emit_gd(i - 2, first=(i == 2), last=False)

                    ex = exp_.tile([128, 2, 2, 128], F32)
                    nc.scalar.activation(ex[:], p1v[:, :, 1:3, :], EXP)
                    at = atp.tile([128, 2, 128], F16)
                    nc.scalar.copy(at[:], p1v[:, :, 0, :])
                    ats[i] = at
                    sums = stp.tile([128, 2, 2], F32, tag="sums")
                    nc.vector.reduce_sum(
                        sums[:], ex[:], axis=mybir.AxisListType.X
                    )
                    rec = stp.tile([128, 2, 2], F32, tag="rec")
                    nc.vector.reciprocal(rec[:], sums[:])
                    nc.vector.tensor_mul(
                        mwts[i][:],
                        ex[:],
                        rec[:, :, :, None].broadcast_to([128, 2, 2, 128]),
                    )

                emit_gd(NP - 2, first=(NP == 2), last=False)
                emit_gd(NP - 1, first=False, last=True)
                gdts = gdsp.tile([128, 128], F16)
                nc.vector.tensor_copy(gdts[:], gdt[:])

                # ---- phase 3: transpose W, distribute, reconstruct ----
                wsbs = []
                osb = None

                def emit_group(g, osb_t):
                    zun = pZO.tile([128, 512], F32, tag="pzo", name=f"zun{s}_{g}")
                    nc.tensor.matmul(
                        zun[:], gdts[:], wsbs[g][:], start=True, stop=True
                    )
                    zsb = zsbp.tile([128, 512], F16, name=f"zsb{s}_{g}")
                    nc.scalar.copy(zsb[:], zun[:])
                    half = (g % 2) * 512
                    for k in range(KC):
                        ock = pZO.tile(
                            [128, 512], F32, tag="pzo", name=f"ock{s}_{g}_{k}"
                        )
                        nc.tensor.matmul(
                            ock[:], wrt[:, k * 128:(k + 1) * 128], zsb[:],
                            start=True, stop=True,
                        )
                        dst = osb_t[:, k, half:half + 512]
                        if has_bias_r:
                            nc.scalar.activation(
                                dst, ock[:], IDENT, bias=brt[:, k:k + 1]
                            )
                        elif k % 2 == 0:
                            nc.scalar.copy(dst, ock[:])
                        else:
                            nc.vector.tensor_copy(dst, ock[:])
                    if g % 2 == 1:
                        dst = out_d[s].rearrange("(k p) n -> p k n", p=128)
                        c0 = (g - 1) * 512
                        nc.sync.dma_start(dst[:, :, c0:c0 + 1024], osb_t[:])

                for g in range(NG):
                    wpt = pW.tile([128, 512], F16)
                    for j in range(KC):
                        nt = 4 * g + j
                        nc.tensor.transpose(
                            wpt[:, j * 128:(j + 1) * 128],
                            mwts[nt // 2][:, nt % 2, 1, :],
                            ident16[:],
                        )
                    wsb = wsbp.tile([128, 512], F16, tag="wsb", name=f"wsb{s}_{g}")
                    nc.vector.tensor_copy(wsb[:], wpt[:])
                    wsbs.append(wsb)

                    if g % 2 == 0:
                        osb = osbp.tile(
                            [128, KC, 1024], F16, tag="osb", name=f"osb{s}_{g}"
                        )
                    if g > 0:
                        emit_group(g - 1, osb if g % 2 == 1 else prev_osb)
                    prev_osb = osb
                emit_group(NG - 1, osb)

    nc.compile()
    return nc


_CACHE = {}


def _get_nc(has_bias_abv: bool, has_bias_r: bool):
    key = (has_bias_abv, has_bias_r)
    if key not in _CACHE:
        _CACHE[key] = _build(*key)
    return _CACHE[key]


def _run(inputs, trace=False, **spmd_kwargs):
    x = np.asarray(inputs["x"])
    b, c, h, w = x.shape
    assert (b, c, h, w) == (B_GLOBAL, C_IN, H, W), x.shape
    wA = np.asarray(inputs["wA"], np.float32)
    wB = np.asarray(inputs["wB"], np.float32)
    wV = np.asarray(inputs["wV"], np.float32)
    wR = np.asarray(inputs["wR"], np.float32)
    bA = np.asarray(inputs["bA"], np.float32)
    bB = np.asarray(inputs["bB"], np.float32)
    bV = np.asarray(inputs["bV"], np.float32)
    bR = np.asarray(inputs["bR"], np.float32)

    has_bias_abv = bool(np.any(bA) or np.any(bB) or np.any(bV))
    has_bias_r = bool(np.any(bR))
    nc = _get_nc(has_bias_abv, has_bias_r)

    # [KC, 128, 384] : chunk k holds [wA.T | wB.T | wV.T][k*128:(k+1)*128, :]
    wcat = np.concatenate([wA.T, wB.T, wV.T], axis=1).reshape(KC, 128, 3 * 128)
    base = {
        "wcat": np.ascontiguousarray(wcat, dtype=np.float16),
        "wrt": np.ascontiguousarray(wR.T, dtype=np.float16),
    }
    if has_bias_abv:
        base["bcat"] = np.concatenate([bA, bB, bV])[None, :].astype(np.float16)
    if has_bias_r:
        base["brt"] = np.ascontiguousarray(bR.reshape(KC, 128).T, np.float32)

    xf = np.asarray(x, np.float16).reshape(B_GLOBAL, C_IN, N)
    in_maps = [
        dict(base, x=np.ascontiguousarray(xf[ci * B_LOC:(ci + 1) * B_LOC]))
        for ci in range(N_CORES)
    ]
    res = run_bass_kernel_spmd(
        nc, in_maps, core_ids=list(range(N_CORES)), trace=trace, **spmd_kwargs
    )
    out = np.concatenate(
        [res.results[ci]["out"].astype(np.float32) for ci in range(N_CORES)],
        axis=0,
    )
    return out.reshape(B_GLOBAL, C_IN, H, W), res


def kernel(**inputs):
    out, _ = _run(inputs)
    return out
